# revision 13
# baseline (speedup 1.0000x reference)
"""Trainium2 Bass kernel for nn_DistanceLoss (patch neighbor-distance loss).

Reference semantics (k=16, H=W=2048, LOSS_WEIGHT=1):
  split each image into non-overlapping 16x16 patches; for interior pixels
  (local i,j in 1..14) and the 8-neighbor offset list [E,NW,NE,N,E,SW,SE,S]
  (E twice, W missing), accumulate || |sr_c-sr_n| - |hr_c-hr_n| || and take
  the global mean over L*14*14*8 terms.

Identity: for u = sr_c-sr_n, v = hr_c-hr_n,
    ||u|-|v|| = min(|u+v|, |u-v|) = min(|S_c-S_n|, |D_c-D_n|)
with S = sr+hr, D = sr-hr. Opposite offsets +o/-o share one difference
array t: the pairs {N,S}, {NW,SE}, {NE,SW} cost one elementwise pass each;
E (listed twice) has weight 2.

Sharding: 256 image columns per core (16 patch-cols x 128 patch-rows).
Host reshapes each slab to [128, 4096] (partition = patch-row, free =
i*256+c) making every neighbor offset the constant free shift di*256+dj.

Measured-HW design notes (bench on the target trn2):
  - DVE TT fp16 runs 2x even with ODD element offsets, so shifted operands
    SD[o:...] are sliced directly; no SBUF->SBUF shifted-copy DMA at all.
  - STT/TensorReduce run at 1x -> no fused accumulate paths; reductions
    stay on the otherwise-idle PE as ones/twos-weighted [128,1]^T @ t-row
    matmuls into one PSUM region (row weights {1,2,...,2,1} encode both
    shifted windows of an offset pair, strips are edge columns, E bakes
    its x2). Same-weight adjacent rows batch 2-per-matmul (448 <= 512
    moving limit).
  - Everything is processed in row-halves (i rows 0..7 | 8..14): TT, abs,
    min, and the PE row-matmuls pipeline at half-tile granularity, so PE
    starts reducing a pair as soon as its first min-half lands and the
    final PE tail is only the last half of the E pair.
  - abs: ACT Abs (0.87ns/elem) takes the three 256/255/257 pairs
    (in-place halves on the stacked p|q tile); the E pair's abs rides
    DVE int16 sign-clear at 4x. Balances DVE ~27us / ACT ~20us.
  - input DMA: fp16, 6 chunks/tensor on parallel queues, small first
    chunks so S|D prep and the first pair-TT halves start early.
"""

import numpy as np

H = W = 2048
K = 16
NCORES = 8
WC = W // NCORES          # 256 columns per core
FREE = K * WC             # 4096 free elements per partition
WIN = 15 * WC             # 3840: compute window covers i = 0..14
SEG = FREE + 64           # per-segment pad: o=257 shifted reads end at 4097
HALF = 2048               # row-half split: rows 0..7 | 8..14
N_TERMS = (H // K) * (W // K) * (K - 2) * (K - 2) * 8


def _split_multiwaits(nc):
    """The walrus build here accepts at most one sync wait (and one update)
    per instruction: hoist extra waits onto same-engine NoOps inserted
    before the instruction, and extra updates onto NoOps after it."""
    from concourse import mybir

    k = 0
    for f in nc.m.functions:
        for bb in f.blocks:
            out, changed = [], False
            for i in bb.instructions:
                si = i.sync_info
                waits = list(si.on_wait) if si else []
                ups = list(si.on_update) if si else []
                trimmed = False
                if len(waits) > 1:
                    for w in waits[:-1]:
                        n = mybir.InstNoOp(name=f"{i.name}-sw{k}", ins=[],
                                           outs=[])
                        k += 1
                        n.engine = i.engine
                        n.sync_info = mybir.SyncInfo(on_wait=[w], on_update=[])
                        out.append(n)
                    waits, changed, trimmed = waits[-1:], True, True
                out.append(i)
                if len(ups) > 1:
                    i.sync_info = mybir.SyncInfo(on_wait=waits,
                                                 on_update=ups[:1])
                    for u in ups[1:]:
                        n = mybir.InstNoOp(name=f"{i.name}-su{k}", ins=[],
                                           outs=[])
                        k += 1
                        n.engine = i.engine
                        n.sync_info = mybir.SyncInfo(on_wait=[], on_update=[u])
                        out.append(n)
                    changed = True
                elif trimmed:
                    i.sync_info = mybir.SyncInfo(on_wait=waits, on_update=ups)
            if changed:
                bb.instructions = out
    return k


def _build_bass(debug=False):
    from concourse import bass, mybir, tile

    nc = bass.Bass()
    x_sr = nc.declare_dram_parameter("x_sr", [128, FREE], mybir.dt.float16,
                                     isOutput=False)
    x_hr = nc.declare_dram_parameter("x_hr", [128, FREE], mybir.dt.float16,
                                     isOutput=False)
    out_sum = nc.declare_dram_parameter("out_sum", [1, 8],
                                        mybir.dt.float32, isOutput=True)
    dbg_t = None
    if debug:
        dbg_t = [nc.declare_dram_parameter(f"dbg_t{k}", [128, WIN],
                                           mybir.dt.float16, isOutput=True)
                 for k in range(4)]

    fp16 = mybir.dt.float16
    f32 = mybir.dt.float32
    Alu = mybir.AluOpType
    Act = mybir.ActivationFunctionType

    with tile.TileContext(nc) as tc:
        with tc.tile_pool(name="io", bufs=1) as io_pool, \
             tc.tile_pool(name="sd", bufs=1) as sd_pool, \
             tc.tile_pool(name="pq", bufs=3) as pq_pool, \
             tc.tile_pool(name="tpool", bufs=4) as t_pool, \
             tc.tile_pool(name="psum", bufs=1, space="PSUM") as psum_pool:
            sr_t = io_pool.tile([128, FREE], fp16, tag="sr")
            hr_t = io_pool.tile([128, FREE], fp16, tag="hr")
            SD = sd_pool.tile([128, 2 * SEG], fp16, tag="SD")
            w1 = sd_pool.tile([128, 1], fp16, tag="w1")
            w2 = sd_pool.tile([128, 1], fp16, tag="w2")
            wm2 = sd_pool.tile([128, 1], fp16, tag="wm2")
            w2f = sd_pool.tile([128, 1], f32, tag="w2f")
            racc = sd_pool.tile([128, 2], f32, tag="racc")
            acc = psum_pool.tile([1, 512], f32, tag="acc")
            colsb = sd_pool.tile([1, 8], f32, tag="colsb")

            SDv = SD.rearrange("p (s f) -> p s f", s=2)

            nc.vector.memset(w1[:, :], 1.0)
            nc.vector.memset(w2[:, :], 2.0)
            nc.vector.memset(wm2[:, :], -2.0)
            nc.vector.memset(w2f[:, :], 2.0)
            # shifted reads run into the per-segment pad; keep it defined
            nc.vector.memset(SDv[:, :, FREE:], 0.0)

            # chunked fp16 input loads: 3 big chunks per tensor (large
            # transfers amortize the SDMA descriptor overhead; one DMA
            # already spreads across all 16 SDMA engines) with the issue
            # spread across three DGE paths so descriptors go out in
            # parallel instead of serializing ~0.6us apiece on Sync
            bounds = [0, 1024, 2560, FREE]
            for c in range(len(bounds) - 1):
                lo, hi = bounds[c], bounds[c + 1]
                nc.sync.dma_start(out=sr_t[:, lo:hi], in_=x_sr[:, lo:hi])
                nc.scalar.dma_start(out=hr_t[:, lo:hi], in_=x_hr[:, lo:hi])
            # S|D prep per chunk (S=sr+hr, D=sr-hr)
            for c in range(len(bounds) - 1):
                lo, hi = bounds[c], bounds[c + 1]
                nc.vector.tensor_tensor(SDv[:, 0, lo:hi], sr_t[:, lo:hi],
                                        hr_t[:, lo:hi], Alu.add)
                nc.vector.tensor_tensor(SDv[:, 1, lo:hi], sr_t[:, lo:hi],
                                        hr_t[:, lo:hi], Alu.subtract)

            # Per-pair plans. Row tasks: (row, jlo, jhi, weight); strips
            # are single-window edge columns emitted as one matmul per
            # row-half. Weights {1,2,...,2,1} over rows 0..14 encode the
            # two shifted windows of each +o/-o pair; E bakes its x2.
            def midrows(jlo, jhi):
                return [(i, jlo, jhi, 1 if i in (0, 14) else 2)
                        for i in range(15)]

            PAIRS = [
                # o=256 {N,S}: rows 0..14 weighted, j 1..14
                (256, 0, "act", midrows(1, 15), [], False),
                # o=255 {NE,SW}: mid j 2..14 + edge cols j=1 (rows 1..14),
                # j=15 (rows 0..13)
                (255, 0, "act", midrows(2, 15), [(1, 1, 15), (15, 0, 14)],
                 False),
                # o=257 {NW,SE}: mid j 1..13 + edge cols j=14 (rows 1..14),
                # j=0 (rows 0..13)
                (257, 0, "act", midrows(1, 14), [(14, 1, 15), (0, 0, 14)],
                 True),
                # E (o=1, weight 2): rows 1..14, j 1..14 -- reduced on
                # DVE via 4x tensor_scalar accumulate; PE only subtracts
                # the excluded j=0,15 columns with weight -2
                (1, WC, "dve", "ts", [], True),
            ]

            first_mm = [True]

            def mm(rhs, wts, stop=False):
                width = int(np.prod(rhs.shape[1:]))
                nc.tensor.matmul(acc[:, 0:width], wts[:, :], rhs,
                                 start=first_mm[0], stop=stop)
                first_mm[0] = False

            n_pairs = len(PAIRS)
            for pi, (o, oplo, abs_eng, rows, strips, split) in \
                    enumerate(PAIRS):
                last_pair = pi == n_pairs - 1
                pq = pq_pool.tile([128, 2 * WIN], fp16, tag="pq")
                t_a = t_pool.tile([128, HALF], fp16, tag="ta")
                t_b = t_pool.tile([128, WIN - HALF], fp16, tag="tb")
                pqv = pq.rearrange("p (s f) -> p s f", s=2)
                vza = t_a.rearrange("p (i q j) -> p i q j", q=16, j=16)
                vzb = t_b.rearrange("p (i q j) -> p i q j", q=16, j=16)

                halves = [(oplo, HALF), (HALF, WIN)]
                tt_parts = halves if split else [(oplo, WIN)]
                for hlo, hhi in tt_parts:
                    # p|q = SD - SD[o:] (odd offsets slice SD directly;
                    # 2x TT confirmed on HW for odd element offsets)
                    nc.vector.tensor_tensor(pqv[:, :, hlo:hhi],
                                            SDv[:, :, hlo:hhi],
                                            SDv[:, :, o + hlo:o + hhi],
                                            Alu.subtract)
                for hlo, hhi in halves:
                    # |pq| in place: ACT Abs for the three big pairs,
                    # DVE int16 sign-clear (4x) for the E pair
                    if abs_eng == "act":
                        nc.scalar.activation(pqv[:, :, hlo:hhi],
                                             pqv[:, :, hlo:hhi], Act.Abs)
                    else:
                        pqi = pqv[:, :, hlo:hhi].bitcast(mybir.dt.int16)
                        nc.vector.tensor_scalar(out=pqi, in0=pqi,
                                                scalar1=0x7FFF, scalar2=None,
                                                op0=Alu.bitwise_and)
                for hi_, (hlo, hhi) in enumerate(halves):
                    # t = min(|p|, |q|) into the row-half tile
                    dst = t_a[:, hlo:hhi] if hi_ == 0 else t_b[:, 0:hhi - hlo]
                    nc.vector.tensor_tensor(dst, pq[:, hlo:hhi],
                                            pq[:, WIN + hlo:WIN + hhi],
                                            Alu.min)
                    vz = vza if hi_ == 0 else vzb
                    base = 0 if hi_ == 0 else 8
                    if rows == "ts":
                        # DVE 4x flat accumulate of this half's window
                        # rows (weight applied later via the racc matmul)
                        tsrc = dst
                        tsi = tsrc.bitcast(fp16)
                        nc.vector.tensor_scalar(
                            out=tsi, in0=tsi, scalar1=1.0, scalar2=0.0,
                            op0=Alu.mult, op1=Alu.add,
                            accum_out=racc[:, hi_:hi_ + 1])
                        # PE: subtract excluded j in {0,15} columns, w=-2
                        rl = 1 if hi_ == 0 else 8
                        rh2 = 8 if hi_ == 0 else 15
                        for j in (0, 15):
                            mm(vz[:, rl - base:rh2 - base, :, j:j + 1], wm2)
                        if hi_ == 1:
                            # fold the two flat sums in with weight +2;
                            # this is the final matmul of the kernel
                            mm(racc[:, 0:2], w2f, stop=True)
                        continue
                    # PE row reductions for this half, batching adjacent
                    # same-weight rows two per matmul (width <= 448) and
                    # grouping same-stationary matmuls (w2 first, then
                    # w1 + strips) to minimize LDWEIGHTS reloads
                    hrows = [r for r in rows
                             if (r[0] < 8) == (hi_ == 0)]
                    w2_mms, w1_mms = [], []
                    bi = 0
                    while bi < len(hrows):
                        r0 = hrows[bi]
                        batch = [r0]
                        if (bi + 1 < len(hrows)
                                and hrows[bi + 1][0] == r0[0] + 1
                                and hrows[bi + 1][1:] == r0[1:]):
                            batch.append(hrows[bi + 1])
                        bi += len(batch)
                        i0 = r0[0] - base
                        rhs = vz[:, i0:i0 + len(batch), :, r0[1]:r0[2]]
                        (w1_mms if r0[3] == 1 else w2_mms).append(rhs)
                    for j, rlo, rhi in strips:
                        lo = max(rlo, 0 if hi_ == 0 else 8)
                        hi2 = min(rhi, 8 if hi_ == 0 else 15)
                        if lo >= hi2:
                            continue
                        w1_mms.append(vz[:, lo - base:hi2 - base, :,
                                         j:j + 1])
                    for rhs in w2_mms:
                        mm(rhs, w2)
                    for rhs in w1_mms:
                        mm(rhs, w1)
                if debug:
                    nc.sync.dma_start(out=dbg_t[pi][:, 0:HALF],
                                      in_=t_a[:, 0:HALF])
                    nc.sync.dma_start(out=dbg_t[pi][:, HALF:WIN],
                                      in_=t_b[:, 0:WIN - HALF])

            # drain PSUM to a scalar
            nc.vector.tensor_reduce(colsb[:, 0:1], acc[:, 0:448],
                                    mybir.AxisListType.X, Alu.add)
            nc.sync.dma_start(out=out_sum[:, :], in_=colsb[:, :])
    _split_multiwaits(nc)
    return nc


_NC_CACHE = None
LAST_RESULTS = None  # BassKernelResults of the most recent run (for test.py)


def kernel(sr_tensor: np.ndarray, hr_tensor: np.ndarray) -> np.ndarray:
    from concourse.bass_utils import run_bass_kernel_spmd

    global _NC_CACHE, LAST_RESULTS
    if _NC_CACHE is None:
        _NC_CACHE = _build_bass()
    nc = _NC_CACHE

    # fp16 staging: the kernel computes in fp16 on-device either way; the
    # cast here just halves DMA traffic.
    sr = np.asarray(sr_tensor, dtype=np.float32).reshape(H, W)
    hr = np.asarray(hr_tensor, dtype=np.float32).reshape(H, W)

    in_maps = []
    for c in range(NCORES):
        c0 = c * WC
        # [2048, 256] -> [128 patch-rows, 16 rows, 256 cols] -> [128, 4096]
        slab_sr = np.ascontiguousarray(
            sr[:, c0:c0 + WC].reshape(128, K, WC).reshape(128, FREE)
            .astype(np.float16))
        slab_hr = np.ascontiguousarray(
            hr[:, c0:c0 + WC].reshape(128, K, WC).reshape(128, FREE)
            .astype(np.float16))
        in_maps.append({"x_sr": slab_sr, "x_hr": slab_hr})

    res = run_bass_kernel_spmd(nc, in_maps, list(range(NCORES)))
    LAST_RESULTS = res

    total = 0.0
    for r in res.results:
        total += float(np.asarray(r["out_sum"], dtype=np.float64)[0, 0])
    return np.float32(total / N_TERMS)


# revision 14
# speedup vs baseline: 1.0423x; 1.0423x over previous
"""Trainium2 Bass kernel for nn_DistanceLoss (patch neighbor-distance loss).

Reference semantics (k=16, H=W=2048, LOSS_WEIGHT=1):
  split each image into non-overlapping 16x16 patches; for interior pixels
  (local i,j in 1..14) and the 8-neighbor offset list [E,NW,NE,N,E,SW,SE,S]
  (E twice, W missing), accumulate || |sr_c-sr_n| - |hr_c-hr_n| || and take
  the global mean over L*14*14*8 terms.

Identity: for u = sr_c-sr_n, v = hr_c-hr_n,
    ||u|-|v|| = min(|u+v|, |u-v|) = min(|S_c-S_n|, |D_c-D_n|)
with S = sr+hr, D = sr-hr. Opposite offsets +o/-o share one difference
array t: the pairs {N,S}, {NW,SE}, {NE,SW} cost one elementwise pass each;
E (listed twice) has weight 2.

Sharding: 256 image columns per core (16 patch-cols x 128 patch-rows).
Host reshapes each slab to [128, 4096] (partition = patch-row, free =
i*256+c) making every neighbor offset the constant free shift di*256+dj.

Measured-HW design notes (bench on the target trn2):
  - DVE TT fp16 runs 2x even with ODD element offsets, so shifted operands
    SD[o:...] are sliced directly; no SBUF->SBUF shifted-copy DMA at all.
  - STT/TensorReduce run at 1x -> no fused accumulate paths; reductions
    stay on the otherwise-idle PE as ones/twos-weighted [128,1]^T @ t-row
    matmuls into one PSUM region (row weights {1,2,...,2,1} encode both
    shifted windows of an offset pair, strips are edge columns, E bakes
    its x2). Same-weight adjacent rows batch 2-per-matmul (448 <= 512
    moving limit).
  - Everything is processed in row-halves (i rows 0..7 | 8..14): TT, abs,
    min, and the PE row-matmuls pipeline at half-tile granularity, so PE
    starts reducing a pair as soon as its first min-half lands and the
    final PE tail is only the last half of the E pair.
  - abs: ACT Abs (0.87ns/elem) takes the three 256/255/257 pairs
    (in-place halves on the stacked p|q tile); the E pair's abs rides
    DVE int16 sign-clear at 4x. Balances DVE ~27us / ACT ~20us.
  - input DMA: fp16, 6 chunks/tensor on parallel queues, small first
    chunks so S|D prep and the first pair-TT halves start early.
"""

import numpy as np

H = W = 2048
K = 16
NCORES = 8
WC = W // NCORES          # 256 columns per core
FREE = K * WC             # 4096 free elements per partition
WIN = 15 * WC             # 3840: compute window covers i = 0..14
SEG = FREE + 64           # per-segment pad: o=257 shifted reads end at 4097
HALF = 2048               # row-half split: rows 0..7 | 8..14
N_TERMS = (H // K) * (W // K) * (K - 2) * (K - 2) * 8


def _split_multiwaits(nc):
    """The walrus build here accepts at most one sync wait (and one update)
    per instruction: hoist extra waits onto same-engine NoOps inserted
    before the instruction, and extra updates onto NoOps after it."""
    from concourse import mybir

    k = 0
    for f in nc.m.functions:
        for bb in f.blocks:
            out, changed = [], False
            for i in bb.instructions:
                si = i.sync_info
                waits = list(si.on_wait) if si else []
                ups = list(si.on_update) if si else []
                trimmed = False
                if len(waits) > 1:
                    for w in waits[:-1]:
                        n = mybir.InstNoOp(name=f"{i.name}-sw{k}", ins=[],
                                           outs=[])
                        k += 1
                        n.engine = i.engine
                        n.sync_info = mybir.SyncInfo(on_wait=[w], on_update=[])
                        out.append(n)
                    waits, changed, trimmed = waits[-1:], True, True
                out.append(i)
                if len(ups) > 1:
                    i.sync_info = mybir.SyncInfo(on_wait=waits,
                                                 on_update=ups[:1])
                    for u in ups[1:]:
                        n = mybir.InstNoOp(name=f"{i.name}-su{k}", ins=[],
                                           outs=[])
                        k += 1
                        n.engine = i.engine
                        n.sync_info = mybir.SyncInfo(on_wait=[], on_update=[u])
                        out.append(n)
                    changed = True
                elif trimmed:
                    i.sync_info = mybir.SyncInfo(on_wait=waits, on_update=ups)
            if changed:
                bb.instructions = out
    return k


def _build_bass(debug=False):
    from concourse import bass, mybir, tile

    nc = bass.Bass()
    x_sr = nc.declare_dram_parameter("x_sr", [128, FREE], mybir.dt.float16,
                                     isOutput=False)
    x_hr = nc.declare_dram_parameter("x_hr", [128, FREE], mybir.dt.float16,
                                     isOutput=False)
    out_sum = nc.declare_dram_parameter("out_sum", [1, 8],
                                        mybir.dt.float32, isOutput=True)
    dbg_t = None
    if debug:
        dbg_t = [nc.declare_dram_parameter(f"dbg_t{k}", [128, WIN],
                                           mybir.dt.float16, isOutput=True)
                 for k in range(4)]

    fp16 = mybir.dt.float16
    f32 = mybir.dt.float32
    Alu = mybir.AluOpType
    Act = mybir.ActivationFunctionType

    with tile.TileContext(nc) as tc:
        with tc.tile_pool(name="io", bufs=1) as io_pool, \
             tc.tile_pool(name="sd", bufs=1) as sd_pool, \
             tc.tile_pool(name="pq", bufs=3) as pq_pool, \
             tc.tile_pool(name="tpool", bufs=4) as t_pool, \
             tc.tile_pool(name="psum", bufs=1, space="PSUM") as psum_pool:
            sr_t = io_pool.tile([128, FREE], fp16, tag="sr")
            hr_t = io_pool.tile([128, FREE], fp16, tag="hr")
            SD = sd_pool.tile([128, 2 * SEG], fp16, tag="SD")
            w1 = sd_pool.tile([128, 1], fp16, tag="w1")
            w2 = sd_pool.tile([128, 1], fp16, tag="w2")
            wm2 = sd_pool.tile([128, 1], fp16, tag="wm2")
            w2f = sd_pool.tile([128, 1], f32, tag="w2f")
            racc = sd_pool.tile([128, 2], f32, tag="racc")
            acc = psum_pool.tile([1, 512], f32, tag="acc")
            colsb = sd_pool.tile([1, 8], f32, tag="colsb")

            SDv = SD.rearrange("p (s f) -> p s f", s=2)

            nc.vector.memset(w1[:, :], 1.0)
            nc.vector.memset(w2[:, :], 2.0)
            nc.vector.memset(wm2[:, :], -2.0)
            nc.vector.memset(w2f[:, :], 2.0)
            # shifted reads run into the per-segment pad; keep it defined
            nc.vector.memset(SDv[:, :, FREE:], 0.0)

            # chunked fp16 input loads: 3 big chunks per tensor (large
            # transfers amortize the SDMA descriptor overhead; one DMA
            # already spreads across all 16 SDMA engines) with the issue
            # spread across three DGE paths so descriptors go out in
            # parallel instead of serializing ~0.6us apiece on Sync
            bounds = [0, 512, 1536, 2816, FREE]
            for c in range(len(bounds) - 1):
                lo, hi = bounds[c], bounds[c + 1]
                nc.sync.dma_start(out=sr_t[:, lo:hi], in_=x_sr[:, lo:hi])
                nc.scalar.dma_start(out=hr_t[:, lo:hi], in_=x_hr[:, lo:hi])
            # S|D prep per chunk (S=sr+hr, D=sr-hr)
            for c in range(len(bounds) - 1):
                lo, hi = bounds[c], bounds[c + 1]
                nc.vector.tensor_tensor(SDv[:, 0, lo:hi], sr_t[:, lo:hi],
                                        hr_t[:, lo:hi], Alu.add)
                nc.vector.tensor_tensor(SDv[:, 1, lo:hi], sr_t[:, lo:hi],
                                        hr_t[:, lo:hi], Alu.subtract)

            # Per-pair plans. Row tasks: (row, jlo, jhi, weight); strips
            # are single-window edge columns emitted as one matmul per
            # row-half. Weights {1,2,...,2,1} over rows 0..14 encode the
            # two shifted windows of each +o/-o pair; E bakes its x2.
            def midrows(jlo, jhi):
                return [(i, jlo, jhi, 1 if i in (0, 14) else 2)
                        for i in range(15)]

            PAIRS = [
                # o=256 {N,S}: rows 0..14 weighted, j 1..14
                (256, 0, "act", midrows(1, 15), [], True),
                # o=255 {NE,SW}: mid j 2..14 + edge cols j=1 (rows 1..14),
                # j=15 (rows 0..13)
                (255, 0, "act", midrows(2, 15), [(1, 1, 15), (15, 0, 14)],
                 True),
                # o=257 {NW,SE}: mid j 1..13 + edge cols j=14 (rows 1..14),
                # j=0 (rows 0..13)
                (257, 0, "act", midrows(1, 14), [(14, 1, 15), (0, 0, 14)],
                 True),
                # E (o=1, weight 2): rows 1..14, j 1..14
                (1, WC, "dve",
                 [(i, 1, 15, 2) for i in range(1, 15)], [], True),
            ]

            first_mm = [True]

            def mm(rhs, wts, stop=False):
                width = int(np.prod(rhs.shape[1:]))
                nc.tensor.matmul(acc[:, 0:width], wts[:, :], rhs,
                                 start=first_mm[0], stop=stop)
                first_mm[0] = False

            n_pairs = len(PAIRS)
            for pi, (o, oplo, abs_eng, rows, strips, split) in \
                    enumerate(PAIRS):
                last_pair = pi == n_pairs - 1
                pq = pq_pool.tile([128, 2 * WIN], fp16, tag="pq")
                t_a = t_pool.tile([128, HALF], fp16, tag="ta")
                t_b = t_pool.tile([128, WIN - HALF], fp16, tag="tb")
                pqv = pq.rearrange("p (s f) -> p s f", s=2)
                vza = t_a.rearrange("p (i q j) -> p i q j", q=16, j=16)
                vzb = t_b.rearrange("p (i q j) -> p i q j", q=16, j=16)

                halves = [(oplo, HALF), (HALF, WIN)]
                tt_parts = halves if split else [(oplo, WIN)]
                for hlo, hhi in tt_parts:
                    # p|q = SD - SD[o:] (odd offsets slice SD directly;
                    # 2x TT confirmed on HW for odd element offsets)
                    nc.vector.tensor_tensor(pqv[:, :, hlo:hhi],
                                            SDv[:, :, hlo:hhi],
                                            SDv[:, :, o + hlo:o + hhi],
                                            Alu.subtract)
                for hlo, hhi in halves:
                    # |pq| in place: ACT Abs for the three big pairs,
                    # DVE int16 sign-clear (4x) for the E pair
                    if abs_eng == "act":
                        nc.scalar.activation(pqv[:, :, hlo:hhi],
                                             pqv[:, :, hlo:hhi], Act.Abs)
                    else:
                        pqi = pqv[:, :, hlo:hhi].bitcast(mybir.dt.int16)
                        nc.vector.tensor_scalar(out=pqi, in0=pqi,
                                                scalar1=0x7FFF, scalar2=None,
                                                op0=Alu.bitwise_and)
                for hi_, (hlo, hhi) in enumerate(halves):
                    # t = min(|p|, |q|) into the row-half tile
                    dst = t_a[:, hlo:hhi] if hi_ == 0 else t_b[:, 0:hhi - hlo]
                    nc.vector.tensor_tensor(dst, pq[:, hlo:hhi],
                                            pq[:, WIN + hlo:WIN + hhi],
                                            Alu.min)
                    vz = vza if hi_ == 0 else vzb
                    base = 0 if hi_ == 0 else 8
                    # PE row reductions for this half, batching adjacent
                    # same-weight rows two per matmul (width <= 448)
                    hrows = [r for r in rows
                             if (r[0] < 8) == (hi_ == 0)]
                    bi = 0
                    while bi < len(hrows):
                        r0 = hrows[bi]
                        batch = [r0]
                        if (bi + 1 < len(hrows)
                                and hrows[bi + 1][0] == r0[0] + 1
                                and hrows[bi + 1][1:] == r0[1:]):
                            batch.append(hrows[bi + 1])
                        bi += len(batch)
                        i0 = r0[0] - base
                        rhs = vz[:, i0:i0 + len(batch), :, r0[1]:r0[2]]
                        w = w1 if r0[3] == 1 else w2
                        is_last_mm = (last_pair and hi_ == 1
                                      and bi == len(hrows))
                        mm(rhs, w, stop=is_last_mm and not strips)
                    for j, rlo, rhi in strips:
                        lo = max(rlo, 0 if hi_ == 0 else 8)
                        hi2 = min(rhi, 8 if hi_ == 0 else 15)
                        if lo >= hi2:
                            continue
                        mm(vz[:, lo - base:hi2 - base, :, j:j + 1], w1)
                if debug:
                    nc.sync.dma_start(out=dbg_t[pi][:, 0:HALF],
                                      in_=t_a[:, 0:HALF])
                    nc.sync.dma_start(out=dbg_t[pi][:, HALF:WIN],
                                      in_=t_b[:, 0:WIN - HALF])

            # drain PSUM to a scalar
            nc.vector.tensor_reduce(colsb[:, 0:1], acc[:, 0:448],
                                    mybir.AxisListType.X, Alu.add)
            nc.sync.dma_start(out=out_sum[:, :], in_=colsb[:, :])
    _split_multiwaits(nc)
    return nc


_NC_CACHE = None
LAST_RESULTS = None  # BassKernelResults of the most recent run (for test.py)


def kernel(sr_tensor: np.ndarray, hr_tensor: np.ndarray) -> np.ndarray:
    from concourse.bass_utils import run_bass_kernel_spmd

    global _NC_CACHE, LAST_RESULTS
    if _NC_CACHE is None:
        _NC_CACHE = _build_bass()
    nc = _NC_CACHE

    # fp16 staging: the kernel computes in fp16 on-device either way; the
    # cast here just halves DMA traffic.
    sr = np.asarray(sr_tensor, dtype=np.float32).reshape(H, W)
    hr = np.asarray(hr_tensor, dtype=np.float32).reshape(H, W)

    in_maps = []
    for c in range(NCORES):
        c0 = c * WC
        # [2048, 256] -> [128 patch-rows, 16 rows, 256 cols] -> [128, 4096]
        slab_sr = np.ascontiguousarray(
            sr[:, c0:c0 + WC].reshape(128, K, WC).reshape(128, FREE)
            .astype(np.float16))
        slab_hr = np.ascontiguousarray(
            hr[:, c0:c0 + WC].reshape(128, K, WC).reshape(128, FREE)
            .astype(np.float16))
        in_maps.append({"x_sr": slab_sr, "x_hr": slab_hr})

    res = run_bass_kernel_spmd(nc, in_maps, list(range(NCORES)))
    LAST_RESULTS = res

    total = 0.0
    for r in res.results:
        total += float(np.asarray(r["out_sum"], dtype=np.float64)[0, 0])
    return np.float32(total / N_TERMS)


# revision 15
# speedup vs baseline: 1.0829x; 1.0390x over previous
"""Trainium2 Bass kernel for nn_DistanceLoss (patch neighbor-distance loss).

Reference semantics (k=16, H=W=2048, LOSS_WEIGHT=1):
  split each image into non-overlapping 16x16 patches; for interior pixels
  (local i,j in 1..14) and the 8-neighbor offset list [E,NW,NE,N,E,SW,SE,S]
  (E twice, W missing), accumulate || |sr_c-sr_n| - |hr_c-hr_n| || and take
  the global mean over L*14*14*8 terms.

Identity: for u = sr_c-sr_n, v = hr_c-hr_n,
    ||u|-|v|| = min(|u+v|, |u-v|) = min(|S_c-S_n|, |D_c-D_n|)
with S = sr+hr, D = sr-hr. Opposite offsets +o/-o share one difference
array t: the pairs {N,S}, {NW,SE}, {NE,SW} cost one elementwise pass each;
E (listed twice) has weight 2.

Sharding: 256 image columns per core (16 patch-cols x 128 patch-rows).
Host reshapes each slab to [128, 4096] (partition = patch-row, free =
i*256+c) making every neighbor offset the constant free shift di*256+dj.

Measured-HW design notes (bench on the target trn2):
  - DVE TT fp16 runs 2x even with ODD element offsets, so shifted operands
    SD[o:...] are sliced directly; no SBUF->SBUF shifted-copy DMA at all.
  - STT/TensorReduce run at 1x -> no fused accumulate paths; reductions
    stay on the otherwise-idle PE as ones/twos-weighted [128,1]^T @ t-row
    matmuls into one PSUM region (row weights {1,2,...,2,1} encode both
    shifted windows of an offset pair, strips are edge columns, E bakes
    its x2). Same-weight adjacent rows batch 2-per-matmul (448 <= 512
    moving limit).
  - Everything is processed in row-halves (i rows 0..7 | 8..14): TT, abs,
    min, and the PE row-matmuls pipeline at half-tile granularity, so PE
    starts reducing a pair as soon as its first min-half lands and the
    final PE tail is only the last half of the E pair.
  - abs: ACT Abs (0.87ns/elem) takes the three 256/255/257 pairs
    (in-place halves on the stacked p|q tile); the E pair's abs rides
    DVE int16 sign-clear at 4x. Balances DVE ~27us / ACT ~20us.
  - input DMA: fp16, 6 chunks/tensor on parallel queues, small first
    chunks so S|D prep and the first pair-TT halves start early.
"""

import numpy as np

H = W = 2048
K = 16
NCORES = 8
WC = W // NCORES          # 256 columns per core
FREE = K * WC             # 4096 free elements per partition
WIN = 15 * WC             # 3840: compute window covers i = 0..14
SEG = FREE + 64           # per-segment pad: o=257 shifted reads end at 4097
HALF = 2048               # row-half split: rows 0..7 | 8..14
N_TERMS = (H // K) * (W // K) * (K - 2) * (K - 2) * 8


def _split_multiwaits(nc):
    """The walrus build here accepts at most one sync wait (and one update)
    per instruction: hoist extra waits onto same-engine NoOps inserted
    before the instruction, and extra updates onto NoOps after it."""
    from concourse import mybir

    k = 0
    for f in nc.m.functions:
        for bb in f.blocks:
            out, changed = [], False
            for i in bb.instructions:
                si = i.sync_info
                waits = list(si.on_wait) if si else []
                ups = list(si.on_update) if si else []
                trimmed = False
                if len(waits) > 1:
                    for w in waits[:-1]:
                        n = mybir.InstNoOp(name=f"{i.name}-sw{k}", ins=[],
                                           outs=[])
                        k += 1
                        n.engine = i.engine
                        n.sync_info = mybir.SyncInfo(on_wait=[w], on_update=[])
                        out.append(n)
                    waits, changed, trimmed = waits[-1:], True, True
                out.append(i)
                if len(ups) > 1:
                    i.sync_info = mybir.SyncInfo(on_wait=waits,
                                                 on_update=ups[:1])
                    for u in ups[1:]:
                        n = mybir.InstNoOp(name=f"{i.name}-su{k}", ins=[],
                                           outs=[])
                        k += 1
                        n.engine = i.engine
                        n.sync_info = mybir.SyncInfo(on_wait=[], on_update=[u])
                        out.append(n)
                    changed = True
                elif trimmed:
                    i.sync_info = mybir.SyncInfo(on_wait=waits, on_update=ups)
            if changed:
                bb.instructions = out
    return k


def _build_bass(debug=False):
    from concourse import bass, mybir, tile

    nc = bass.Bass()
    x_sr = nc.declare_dram_parameter("x_sr", [128, FREE], mybir.dt.float16,
                                     isOutput=False)
    x_hr = nc.declare_dram_parameter("x_hr", [128, FREE], mybir.dt.float16,
                                     isOutput=False)
    out_sum = nc.declare_dram_parameter("out_sum", [1, 8],
                                        mybir.dt.float32, isOutput=True)
    dbg_t = None
    if debug:
        dbg_t = [nc.declare_dram_parameter(f"dbg_t{k}", [128, WIN],
                                           mybir.dt.float16, isOutput=True)
                 for k in range(4)]

    fp16 = mybir.dt.float16
    f32 = mybir.dt.float32
    Alu = mybir.AluOpType
    Act = mybir.ActivationFunctionType

    with tile.TileContext(nc) as tc:
        with tc.tile_pool(name="io", bufs=1) as io_pool, \
             tc.tile_pool(name="sd", bufs=1) as sd_pool, \
             tc.tile_pool(name="pq", bufs=3) as pq_pool, \
             tc.tile_pool(name="tpool", bufs=4) as t_pool, \
             tc.tile_pool(name="psum", bufs=1, space="PSUM") as psum_pool:
            sr_t = io_pool.tile([128, FREE], fp16, tag="sr")
            hr_t = io_pool.tile([128, FREE], fp16, tag="hr")
            SD = sd_pool.tile([128, 2 * SEG], fp16, tag="SD")
            w1 = sd_pool.tile([128, 1], fp16, tag="w1")
            w2 = sd_pool.tile([128, 1], fp16, tag="w2")
            wm2 = sd_pool.tile([128, 1], fp16, tag="wm2")
            w2f = sd_pool.tile([128, 1], f32, tag="w2f")
            racc = sd_pool.tile([128, 2], f32, tag="racc")
            acc = psum_pool.tile([1, 512], f32, tag="acc")
            colsb = sd_pool.tile([1, 8], f32, tag="colsb")

            SDv = SD.rearrange("p (s f) -> p s f", s=2)

            nc.vector.memset(w1[:, :], 1.0)
            nc.vector.memset(w2[:, :], 2.0)
            nc.vector.memset(wm2[:, :], -2.0)
            nc.vector.memset(w2f[:, :], 2.0)
            # shifted reads run into the per-segment pad; keep it defined
            nc.vector.memset(SDv[:, :, FREE:], 0.0)

            # chunked fp16 input loads: 3 big chunks per tensor (large
            # transfers amortize the SDMA descriptor overhead; one DMA
            # already spreads across all 16 SDMA engines) with the issue
            # spread across three DGE paths so descriptors go out in
            # parallel instead of serializing ~0.6us apiece on Sync
            bounds = [0, 256, 768, 1536, 2304, 3200, FREE]
            for c in range(len(bounds) - 1):
                lo, hi = bounds[c], bounds[c + 1]
                nc.sync.dma_start(out=sr_t[:, lo:hi], in_=x_sr[:, lo:hi])
                nc.sync.dma_start(out=hr_t[:, lo:hi], in_=x_hr[:, lo:hi])
            # S|D prep per chunk (S=sr+hr, D=sr-hr)
            for c in range(len(bounds) - 1):
                lo, hi = bounds[c], bounds[c + 1]
                nc.vector.tensor_tensor(SDv[:, 0, lo:hi], sr_t[:, lo:hi],
                                        hr_t[:, lo:hi], Alu.add)
                nc.vector.tensor_tensor(SDv[:, 1, lo:hi], sr_t[:, lo:hi],
                                        hr_t[:, lo:hi], Alu.subtract)

            # Per-pair plans. Row tasks: (row, jlo, jhi, weight); strips
            # are single-window edge columns emitted as one matmul per
            # row-half. Weights {1,2,...,2,1} over rows 0..14 encode the
            # two shifted windows of each +o/-o pair; E bakes its x2.
            def midrows(jlo, jhi):
                return [(i, jlo, jhi, 1 if i in (0, 14) else 2)
                        for i in range(15)]

            PAIRS = [
                # o=256 {N,S}: rows 0..14 weighted, j 1..14
                (256, 0, "act", midrows(1, 15), [], True),
                # o=255 {NE,SW}: mid j 2..14 + edge cols j=1 (rows 1..14),
                # j=15 (rows 0..13)
                (255, 0, "act", midrows(2, 15), [(1, 1, 15), (15, 0, 14)],
                 True),
                # o=257 {NW,SE}: mid j 1..13 + edge cols j=14 (rows 1..14),
                # j=0 (rows 0..13)
                (257, 0, "act", midrows(1, 14), [(14, 1, 15), (0, 0, 14)],
                 True),
                # E (o=1, weight 2): rows 1..14, j 1..14
                (1, WC, "dve",
                 [(i, 1, 15, 2) for i in range(1, 15)], [], True),
            ]

            first_mm = [True]

            def mm(rhs, wts, stop=False):
                width = int(np.prod(rhs.shape[1:]))
                nc.tensor.matmul(acc[:, 0:width], wts[:, :], rhs,
                                 start=first_mm[0], stop=stop)
                first_mm[0] = False

            n_pairs = len(PAIRS)
            for pi, (o, oplo, abs_eng, rows, strips, split) in \
                    enumerate(PAIRS):
                last_pair = pi == n_pairs - 1
                pq = pq_pool.tile([128, 2 * WIN], fp16, tag="pq")
                t_a = t_pool.tile([128, HALF], fp16, tag="ta")
                t_b = t_pool.tile([128, WIN - HALF], fp16, tag="tb")
                pqv = pq.rearrange("p (s f) -> p s f", s=2)
                vza = t_a.rearrange("p (i q j) -> p i q j", q=16, j=16)
                vzb = t_b.rearrange("p (i q j) -> p i q j", q=16, j=16)

                halves = [(oplo, HALF), (HALF, WIN)]
                tt_parts = halves if split else [(oplo, WIN)]
                for hlo, hhi in tt_parts:
                    # p|q = SD - SD[o:] (odd offsets slice SD directly;
                    # 2x TT confirmed on HW for odd element offsets)
                    nc.vector.tensor_tensor(pqv[:, :, hlo:hhi],
                                            SDv[:, :, hlo:hhi],
                                            SDv[:, :, o + hlo:o + hhi],
                                            Alu.subtract)
                for hlo, hhi in halves:
                    # |pq| in place: ACT Abs for the three big pairs,
                    # DVE int16 sign-clear (4x) for the E pair
                    if abs_eng == "act":
                        nc.scalar.activation(pqv[:, :, hlo:hhi],
                                             pqv[:, :, hlo:hhi], Act.Abs)
                    else:
                        pqi = pqv[:, :, hlo:hhi].bitcast(mybir.dt.int16)
                        nc.vector.tensor_scalar(out=pqi, in0=pqi,
                                                scalar1=0x7FFF, scalar2=None,
                                                op0=Alu.bitwise_and)
                for hi_, (hlo, hhi) in enumerate(halves):
                    # t = min(|p|, |q|) into the row-half tile
                    dst = t_a[:, hlo:hhi] if hi_ == 0 else t_b[:, 0:hhi - hlo]
                    nc.vector.tensor_tensor(dst, pq[:, hlo:hhi],
                                            pq[:, WIN + hlo:WIN + hhi],
                                            Alu.min)
                    vz = vza if hi_ == 0 else vzb
                    base = 0 if hi_ == 0 else 8
                    # PE row reductions for this half, batching adjacent
                    # same-weight rows two per matmul (width <= 448)
                    hrows = [r for r in rows
                             if (r[0] < 8) == (hi_ == 0)]
                    bi = 0
                    while bi < len(hrows):
                        r0 = hrows[bi]
                        batch = [r0]
                        if (bi + 1 < len(hrows)
                                and hrows[bi + 1][0] == r0[0] + 1
                                and hrows[bi + 1][1:] == r0[1:]):
                            batch.append(hrows[bi + 1])
                        bi += len(batch)
                        i0 = r0[0] - base
                        rhs = vz[:, i0:i0 + len(batch), :, r0[1]:r0[2]]
                        w = w1 if r0[3] == 1 else w2
                        is_last_mm = (last_pair and hi_ == 1
                                      and bi == len(hrows))
                        mm(rhs, w, stop=is_last_mm and not strips)
                    for j, rlo, rhi in strips:
                        lo = max(rlo, 0 if hi_ == 0 else 8)
                        hi2 = min(rhi, 8 if hi_ == 0 else 15)
                        if lo >= hi2:
                            continue
                        mm(vz[:, lo - base:hi2 - base, :, j:j + 1], w1)
                if debug:
                    nc.sync.dma_start(out=dbg_t[pi][:, 0:HALF],
                                      in_=t_a[:, 0:HALF])
                    nc.sync.dma_start(out=dbg_t[pi][:, HALF:WIN],
                                      in_=t_b[:, 0:WIN - HALF])

            # drain PSUM to a scalar
            nc.vector.tensor_reduce(colsb[:, 0:1], acc[:, 0:448],
                                    mybir.AxisListType.X, Alu.add)
            nc.sync.dma_start(out=out_sum[:, :], in_=colsb[:, :])
    _split_multiwaits(nc)
    return nc


_NC_CACHE = None
LAST_RESULTS = None  # BassKernelResults of the most recent run (for test.py)


def kernel(sr_tensor: np.ndarray, hr_tensor: np.ndarray) -> np.ndarray:
    from concourse.bass_utils import run_bass_kernel_spmd

    global _NC_CACHE, LAST_RESULTS
    if _NC_CACHE is None:
        _NC_CACHE = _build_bass()
    nc = _NC_CACHE

    # fp16 staging: the kernel computes in fp16 on-device either way; the
    # cast here just halves DMA traffic.
    sr = np.asarray(sr_tensor, dtype=np.float32).reshape(H, W)
    hr = np.asarray(hr_tensor, dtype=np.float32).reshape(H, W)

    in_maps = []
    for c in range(NCORES):
        c0 = c * WC
        # [2048, 256] -> [128 patch-rows, 16 rows, 256 cols] -> [128, 4096]
        slab_sr = np.ascontiguousarray(
            sr[:, c0:c0 + WC].reshape(128, K, WC).reshape(128, FREE)
            .astype(np.float16))
        slab_hr = np.ascontiguousarray(
            hr[:, c0:c0 + WC].reshape(128, K, WC).reshape(128, FREE)
            .astype(np.float16))
        in_maps.append({"x_sr": slab_sr, "x_hr": slab_hr})

    res = run_bass_kernel_spmd(nc, in_maps, list(range(NCORES)))
    LAST_RESULTS = res

    total = 0.0
    for r in res.results:
        total += float(np.asarray(r["out_sum"], dtype=np.float64)[0, 0])
    return np.float32(total / N_TERMS)
